# revision 5
# baseline (speedup 1.0000x reference)
"""Trainium2 Bass kernel for MemoryWithUsage (scatter_memory).

Computes, per batch b:
  sim = keys @ memory.T * 5 / (|keys| |memory|)   [K, S] cosine attention
  att = softmax(sim, axis=S)
  result = att @ memory                            [K, D]
  usage_r = usage + att.sum(K); loc = argmin(usage_r)
  usage_w = usage_r with usage_w[loc] = 1
  memory_w = decay * memory with row loc replaced by content

Full inputs in, full outputs out; internally sharded 8 batches per core
across 8 NeuronCores (batch-parallel, no cross-core communication).

Layout strategy per core (8 batches, S=8192 as 64 tiles of 128):
 - memory tiles [s=128, d=128] DMA'd natural; PE-transposes build memT
   (extracted PSUM->SBUF by ScalarE); sim via col4-packed matmuls
   (keysT stationary [d,8], memT tiles moving) -> [k@32j, s] PSUM blocks;
   PE transpose-back yields att^T [s, t, k]; softmax denominators via
   ones-vector matmul partition sums; result via col4-packed matmuls
   (att^T tiles stationary, memory tiles moving) + selector-matmul fold.
 - argmin via iota/compare with batched gpsimd partition all-reduces.
 - usage_w written transposed to a scratch output, host reassembles.
 - memory_w = decay*mem written in bulk, content rows scattered by one
   indirect DMA at the end.
"""

import numpy as np

B, S, D, K = 64, 8192, 128, 8
NCORES = 8
BPC = B // NCORES  # 8 batches per core
T = S // 128  # 64 s-tiles per batch
SCALE = 5.0
DECAY = np.float32(1.0 - 1.0 / S)
EPS = 1e-30
BIGIDX = 65536.0

_cache = {}


def _build():
    import concourse.bacc as bacc
    import concourse.bass as bass
    import concourse.tile as tile
    from concourse import bass_isa, masks, mybir

    F32 = mybir.dt.float32
    I32 = mybir.dt.int32
    BF16 = mybir.dt.bfloat16
    ALU = mybir.AluOpType
    AXL = mybir.AxisListType
    ACTF = mybir.ActivationFunctionType

    nc = bacc.Bacc("TRN2", target_bir_lowering=False, debug=False, num_devices=NCORES)

    mem_in = nc.dram_tensor("mem_in", [BPC, S, D], F32, kind="ExternalInput").ap()
    usage_in = nc.dram_tensor("usage_in", [BPC, S], F32, kind="ExternalInput").ap()
    keys_in = nc.dram_tensor("keys_in", [BPC, K, D], F32, kind="ExternalInput").ap()
    content_in = nc.dram_tensor("content_in", [BPC, D], F32, kind="ExternalInput").ap()

    memw_out = nc.dram_tensor("memw_out", [BPC, S, D], F32, kind="ExternalOutput").ap()
    uT_out = nc.dram_tensor("uT_out", [BPC, 128, T], F32, kind="ExternalOutput").ap()
    res_out = nc.dram_tensor("res_out", [BPC, K, D], F32, kind="ExternalOutput").ap()

    def bcast_free(ap, counts):
        """Broadcast a [P, 1] (or [P, n]) AP along extra step-0 free dims.
        counts: list of (step, count) pairs replacing the free dims."""
        return bass.AP(ap.tensor, ap.offset, [ap.ap[0]] + [list(c) for c in counts])

    with tile.TileContext(nc) as tc:
        import contextlib

        ctx = contextlib.ExitStack()
        with ctx:
            consts = ctx.enter_context(tc.tile_pool(name="consts", bufs=1))
            big = ctx.enter_context(tc.tile_pool(name="big", bufs=2))
            med = ctx.enter_context(tc.tile_pool(name="med", bufs=2))
            small = ctx.enter_context(tc.tile_pool(name="small", bufs=2))
            stash = ctx.enter_context(tc.tile_pool(name="stash", bufs=1))
            ps_t = ctx.enter_context(tc.tile_pool(name="ps_t", bufs=2, space="PSUM"))
            ps_big = ctx.enter_context(tc.tile_pool(name="ps_big", bufs=1, space="PSUM"))
            ps_aux = ctx.enter_context(tc.tile_pool(name="ps_aux", bufs=2, space="PSUM"))

            # ---------------- constants (inline data) ----------------
            ident_np = np.eye(128, dtype=np.float32)
            p_idx, t_idx = np.mgrid[0:128, 0:T]
            iota_np = (t_idx * 128 + p_idx).astype(np.float32)
            selm_np = (p_idx[:, 0:K] % 32 == t_idx[:, 0:K]).astype(np.float32)
            base_np = (np.arange(K, dtype=np.float32) * S).reshape(K, 1)

            ident = consts.tile([128, 128], F32)
            nc.sync.dma_start(ident[:], nc.inline_tensor(ident_np, name="identc").ap())
            iota_f = consts.tile([128, T], F32)
            nc.sync.dma_start(iota_f[:], nc.inline_tensor(iota_np, name="iotac").ap())
            iota_m = consts.tile([128, T], F32)
            nc.sync.dma_start(
                iota_m[:], nc.inline_tensor(iota_np - BIGIDX, name="iotamc").ap()
            )
            selm = consts.tile([128, K], F32)
            nc.sync.dma_start(selm[:], nc.inline_tensor(selm_np, name="selmc").ap())
            base_f = consts.tile([K, 1], F32)
            nc.sync.dma_start(base_f[:], nc.inline_tensor(base_np, name="basec").ap())

            onescol = consts.tile([128, 1], F32)
            nc.vector.memset(onescol[:], 1.0)
            decayc = consts.tile([128, 1], F32)
            nc.vector.memset(decayc[:], float(DECAY))
            bigc = consts.tile([128, 1], F32)
            nc.vector.memset(bigc[:], BIGIDX)

            # content rows for final scatter
            content_sb = consts.tile([BPC, D], F32)
            nc.sync.dma_start(content_sb[:], content_in[:])

            # PE warmup (overlaps first DMA): bf16 matmuls on identity
            wps = ps_aux.tile([128, 256], F32, name="warm", tag="aux")
            for i in range(40):
                nc.tensor.matmul(
                    wps[:],
                    ident.bitcast(BF16)[:, 0:128],
                    ident.bitcast(BF16)[:, 0:256],
                    start=(i == 0),
                    stop=(i == 39),
                )

            # ---------------- keys prep ----------------
            keys_sb = consts.tile([B // NCORES * K, D], F32)  # [64, 128]
            nc.sync.dma_start(keys_sb[:], keys_in.rearrange("b k d -> (b k) d"))
            ksq = consts.tile([64, D], F32)
            nc.vector.tensor_tensor(out=ksq[:], in0=keys_sb[:], in1=keys_sb[:], op=ALU.mult)
            kss = consts.tile([64, 1], F32)
            nc.vector.tensor_reduce(out=kss[:], in_=ksq[:], axis=AXL.X, op=ALU.add)
            knorm = consts.tile([64, 1], F32)
            nc.scalar.activation(out=knorm[:], in_=kss[:], func=ACTF.Sqrt)
            kinv = consts.tile([64, 1], F32)
            nc.vector.reciprocal(out=kinv[:], in_=knorm[:])
            # NR refine: kinv = kinv*(2 - knorm*kinv)
            knr = consts.tile([64, 1], F32)
            nc.vector.tensor_tensor(out=knr[:], in0=knorm[:], in1=kinv[:], op=ALU.mult)
            nc.vector.tensor_scalar(
                out=knr[:], in0=knr[:], scalar1=-1.0, scalar2=2.0, op0=ALU.mult, op1=ALU.add
            )
            nc.vector.tensor_tensor(out=kinv[:], in0=kinv[:], in1=knr[:], op=ALU.mult)
            # kscale = 5 * kinv ; keys_scaled = keys * kscale (per-partition)
            nc.vector.tensor_scalar(
                out=kinv[:], in0=kinv[:], scalar1=SCALE, scalar2=None, op0=ALU.mult
            )
            keys_sc = consts.tile([64, D], F32)
            nc.scalar.activation(out=keys_sc[:], in_=keys_sb[:], func=ACTF.Copy, scale=kinv[:])
            # transpose -> keysT [128 d, 64 bk]
            kps = ps_aux.tile([128, 64], F32, name="kps", tag="aux")
            nc.tensor.transpose(kps[:], keys_sc[:], ident[0:64, 0:64])
            keysT = consts.tile([128, 64], F32)
            nc.vector.tensor_copy(keysT[:], kps[:])

            # ---------------- per-batch stashes ----------------
            usageT_all = stash.tile([128, BPC, T], F32)
            usager_all = stash.tile([128, BPC, T], F32)
            rowminneg = stash.tile([128, BPC], F32)
            candneg = stash.tile([128, BPC], F32)
            result_sb = stash.tile([K, BPC, D], F32)

            memw_dmas = []

            # ---------------- main batch loop ----------------
            for b in range(BPC):
                mem = big.tile([128, T, D], F32, name="mem")  # [p, t, d]
                nc.sync.dma_start(
                    mem[:], mem_in[b].rearrange("(t p) d -> p t d", p=128)
                )
                urows = small.tile([T, 128], F32, name="urows")
                nc.sync.dma_start(urows[:], usage_in[b].rearrange("(t p) -> t p", p=128))

                # --- transposes: memT tiles [d, s] ---
                memT = big.tile([128, T, D], F32, name="memT")
                for g in range(8):  # groups of 8 tiles
                    tp = ps_t.tile([128, 8, 128], F32, name="tp")
                    for j in range(8):
                        t = g * 8 + j
                        nc.tensor.transpose(tp[:, j, :], mem[:, t, :], ident[:])
                    nc.scalar.copy(memT[:, g * 8 : g * 8 + 8, :], tp[:])

                # --- norms: ss[p, t] = sum_d mem^2 ---
                ss = med.tile([128, T], F32, name="ss")
                for c in range(4):
                    sq = med.tile([128, 16, D], F32, name="sq")
                    nc.vector.tensor_tensor(
                        out=sq[:],
                        in0=mem[:, 16 * c : 16 * c + 16, :],
                        in1=mem[:, 16 * c : 16 * c + 16, :],
                        op=ALU.mult,
                    )
                    nc.vector.tensor_reduce(
                        out=ss[:, 16 * c : 16 * c + 16], in_=sq[:], axis=AXL.X, op=ALU.add
                    )
                # meminv = 1/(eps + sqrt(ss)), NR-refined against rsqrt
                mnorm = med.tile([128, T], F32, name="mnorm")
                nc.scalar.activation(out=mnorm[:], in_=ss[:], func=ACTF.Sqrt)
                minv = med.tile([128, T], F32, name="minv")
                nc.vector.reciprocal(out=minv[:], in_=mnorm[:])
                mnr = med.tile([128, T], F32, name="mnr")
                nc.vector.tensor_tensor(out=mnr[:], in0=mnorm[:], in1=minv[:], op=ALU.mult)
                nc.vector.tensor_scalar(
                    out=mnr[:], in0=mnr[:], scalar1=-1.0, scalar2=2.0,
                    op0=ALU.mult, op1=ALU.add,
                )
                nc.vector.tensor_tensor(out=minv[:], in0=minv[:], in1=mnr[:], op=ALU.mult)

                # --- sim + transpose-back + exp, in two halves of 32 tiles ---
                att = med.tile([128, T, K], F32, name="att")  # compact [s, t, k]
                for h in range(2):
                    simp = ps_big.tile([128, 8, 128], F32, name="pbig")
                    for g in range(8):  # 8 blocks of 4 tiles
                        for j in range(4):
                            t = h * 32 + g * 4 + j
                            nc.tensor.matmul(
                                simp[32 * j : 32 * j + 8, g, :],
                                keysT[:, 8 * b : 8 * b + 8],
                                memT[:, t, :],
                                tile_position=(0, 32 * j),
                            )
                    sim_sb = med.tile([128, 8, 128], F32, name="sim_sb")
                    nc.vector.tensor_copy(sim_sb[:], simp[:])
                    tbp = ps_big.tile([128, 8, 128], F32, name="pbig")
                    for g in range(8):
                        nc.tensor.transpose(tbp[:, g, :], sim_sb[:, g, :], ident[:])
                    # scaled extract (valid cols only): att_raw = tbp * minv[t]
                    # tbp cols: (g, 32j + k) -> t = h*32 + 4g + j
                    tbp_v = tbp.rearrange("p g (j x) -> p g j x", j=4)[:, :, :, 0:K]
                    minv_b = bass.AP(
                        minv.tensor,
                        minv.offset + h * 32,
                        [minv.ap[0], [4, 8], [1, 4], [0, K]],
                    )
                    nc.vector.tensor_tensor(
                        out=att[:, h * 32 : h * 32 + 32, :].rearrange(
                            "p (g j) k -> p g j k", j=4
                        ),
                        in0=tbp_v,
                        in1=minv_b,
                        op=ALU.mult,
                    )
                nc.scalar.activation(
                    out=att.rearrange("p t k -> p (t k)"),
                    in_=att.rearrange("p t k -> p (t k)"),
                    func=ACTF.Exp,
                )

                # --- softmax denominators: colsum over s-partitions and t ---
                osum = ps_aux.tile([1, T * K], F32, name="osum", tag="aux")
                nc.tensor.matmul(
                    osum[:], onescol[:], att.rearrange("p t k -> p (t k)")
                )
                sums = small.tile([1, K], F32, name="sums")
                nc.vector.tensor_reduce(
                    out=sums[:],
                    in_=bass.AP(
                        osum.tensor, osum.offset, [osum.ap[0], [1, K], [K, T]]
                    ),
                    axis=AXL.X,
                    op=ALU.add,
                )
                inv = small.tile([1, K], F32, name="inv")
                nc.vector.reciprocal(out=inv[:], in_=sums[:])
                snr = small.tile([1, K], F32, name="snr")
                nc.vector.tensor_tensor(out=snr[:], in0=sums[:], in1=inv[:], op=ALU.mult)
                nc.vector.tensor_scalar(
                    out=snr[:], in0=snr[:], scalar1=-1.0, scalar2=2.0,
                    op0=ALU.mult, op1=ALU.add,
                )
                nc.vector.tensor_tensor(out=inv[:], in0=inv[:], in1=snr[:], op=ALU.mult)
                invb = small.tile([128, K], F32, name="invb")
                nc.gpsimd.partition_broadcast(invb[:], inv[:])
                # att_norm = att * invb  (bcast over t)
                nc.vector.tensor_tensor(
                    out=att[:],
                    in0=att[:],
                    in1=bcast_free(invb, [(0, T), (invb.ap[1][0], K)]),
                    op=ALU.mult,
                )

                # --- usage_r and argmin prep ---
                ksum = med.tile([128, T], F32, name="ksum")
                nc.vector.tensor_reduce(out=ksum[:], in_=att[:], axis=AXL.X, op=ALU.add)
                up = ps_aux.tile([128, T], F32, name="up", tag="aux")
                nc.tensor.transpose(up[:, 0:64], urows[:], ident[0:64, 0:64])
                usageT = usageT_all[:, b, :]
                nc.vector.tensor_copy(usageT, up[:, 0:64])
                usager = usager_all[:, b, :]
                nc.vector.tensor_tensor(out=usager, in0=usageT, in1=ksum[:], op=ALU.add)
                rmn = small.tile([128, 1], F32, name="rmn")
                nc.vector.tensor_reduce(out=rmn[:], in_=usager, axis=AXL.X, op=ALU.min)
                nc.vector.tensor_scalar(
                    out=rowminneg[:, b : b + 1], in0=rmn[:], scalar1=-1.0,
                    scalar2=None, op0=ALU.mult,
                )

                # --- result: col4 matmuls, accumulate over t ---
                rps = ps_aux.tile([128, 128], F32, name="rps", tag="aux")
                for t in range(T):
                    j = t % 4
                    nc.tensor.matmul(
                        rps[32 * j : 32 * j + 8, :],
                        att[:, t, :],
                        mem[:, t, :],
                        start=(t < 4),
                        stop=(t >= T - 4),
                        tile_position=(0, 32 * j),
                    )
                res_blk = small.tile([128, 128], F32, name="res_blk")
                nc.vector.tensor_copy(res_blk[:], rps[:])
                fps = ps_aux.tile([K, 128], F32, name="fps", tag="aux")
                nc.tensor.matmul(fps[:], selm[:], res_blk[:])
                nc.vector.tensor_copy(result_sb[:, b, :], fps[:])

                # --- decay in place + bulk writeback ---
                for c in range(4):
                    nc.vector.tensor_scalar(
                        out=mem[:, 16 * c : 16 * c + 16, :],
                        in0=mem[:, 16 * c : 16 * c + 16, :],
                        scalar1=decayc[:],
                        scalar2=None,
                        op0=ALU.mult,
                    )
                dma = nc.sync.dma_start(
                    memw_out[b].rearrange("(t p) d -> p t d", p=128), mem[:]
                )
                memw_dmas.append(dma)

            # ---------------- argmin finale ----------------
            ar1 = stash.tile([128, BPC], F32)
            nc.gpsimd.partition_all_reduce(
                ar1[:], rowminneg[:], channels=128, reduce_op=bass_isa.ReduceOp.max
            )
            for b in range(BPC):
                gmin = small.tile([128, 1], F32, name="gmin")
                nc.vector.tensor_scalar(
                    out=gmin[:], in0=ar1[:, b : b + 1], scalar1=-1.0,
                    scalar2=None, op0=ALU.mult,
                )
                msk = med.tile([128, T], F32, name="msk")
                nc.vector.tensor_scalar(
                    out=msk[:], in0=usager_all[:, b, :], scalar1=gmin[:],
                    scalar2=None, op0=ALU.is_equal,
                )
                cand = med.tile([128, T], F32, name="cand")
                nc.vector.tensor_tensor(out=cand[:], in0=msk[:], in1=iota_m[:], op=ALU.mult)
                nc.vector.tensor_tensor(
                    out=cand[:], in0=cand[:],
                    in1=bcast_free(bigc, [(0, T)]), op=ALU.add,
                )
                cmn = small.tile([128, 1], F32, name="cmn")
                nc.vector.tensor_reduce(out=cmn[:], in_=cand[:], axis=AXL.X, op=ALU.min)
                nc.vector.tensor_scalar(
                    out=candneg[:, b : b + 1], in0=cmn[:], scalar1=-1.0,
                    scalar2=None, op0=ALU.mult,
                )
            ar2 = stash.tile([128, BPC], F32)
            nc.gpsimd.partition_all_reduce(
                ar2[:], candneg[:], channels=128, reduce_op=bass_isa.ReduceOp.max
            )
            # s* per batch (f32): sstar = -ar2
            sstar = stash.tile([128, BPC], F32)
            nc.vector.tensor_scalar(
                out=sstar[:], in0=ar2[:], scalar1=-1.0, scalar2=None, op0=ALU.mult
            )

            # usage_w fixup + write out (transposed scratch)
            for b in range(BPC):
                m2 = med.tile([128, T], mybir.dt.uint8, name="m2")
                nc.vector.tensor_scalar(
                    out=m2[:], in0=iota_f[:], scalar1=sstar[:, b : b + 1],
                    scalar2=None, op0=ALU.is_equal,
                )
                uf = med.tile([128, T], F32, name="uf")
                nc.vector.select(
                    out=uf[:], mask=m2[:],
                    on_true=bcast_free(onescol, [(0, T)]),
                    on_false=usager_all[:, b, :],
                )
                nc.sync.dma_start(uT_out[b], uf[:])

            # result out
            nc.sync.dma_start(
                res_out.rearrange("b k d -> k b d"), result_sb[:]
            )

            # content scatter: global row idx = b*S + s*
            diag = stash.tile([K, BPC], F32)
            nc.vector.tensor_tensor(
                out=diag[:], in0=sstar[0:K, :], in1=ident[0:K, 0:BPC], op=ALU.mult
            )
            grow = stash.tile([K, 1], F32)
            nc.vector.tensor_reduce(out=grow[:], in_=diag[:], axis=AXL.X, op=ALU.add)
            nc.vector.tensor_tensor(out=grow[:], in0=grow[:], in1=base_f[:], op=ALU.add)
            gidx = stash.tile([K, 1], I32)
            nc.vector.tensor_copy(gidx[:], grow[:])
            scat = nc.gpsimd.indirect_dma_start(
                out=memw_out.rearrange("b s d -> (b s) d"),
                out_offset=bass.IndirectOffsetOnAxis(ap=gidx[:, 0:1], axis=0),
                in_=content_sb[:],
                in_offset=None,
            )
            for dma in memw_dmas:
                tile.add_dep_helper(scat.ins, dma.ins, sync=True)

    nc.compile()
    return nc


def _get_nc():
    if "nc" not in _cache:
        _cache["nc"] = _build()
    return _cache["nc"]


def kernel(memory, usage, keys, content):
    from concourse.bass_utils import run_bass_kernel_spmd

    nc = _get_nc()
    memory = np.ascontiguousarray(memory, dtype=np.float32)
    usage = np.ascontiguousarray(usage, dtype=np.float32)
    keys = np.ascontiguousarray(keys, dtype=np.float32)
    content = np.ascontiguousarray(content, dtype=np.float32)

    in_maps = []
    for c in range(NCORES):
        sl = slice(c * BPC, (c + 1) * BPC)
        in_maps.append(
            {
                "mem_in": memory[sl],
                "usage_in": usage[sl],
                "keys_in": keys[sl],
                "content_in": content[sl],
            }
        )
    res = run_bass_kernel_spmd(nc, in_maps, core_ids=list(range(NCORES)))
    _cache["last_result"] = res

    result = np.concatenate([r["res_out"] for r in res.results], axis=0)
    memory_w = np.concatenate([r["memw_out"] for r in res.results], axis=0)
    # uT_out[b] is [128(p), 64(t)]; usage_w[b, t*128+p] = uT[p, t]
    usage_w = np.concatenate(
        [r["uT_out"].transpose(0, 2, 1).reshape(BPC, S) for r in res.results], axis=0
    )
    return result, memory_w, usage_w


# revision 7
# speedup vs baseline: 1.0100x; 1.0100x over previous
"""Trainium2 Bass kernel for MemoryWithUsage (scatter_memory).

Computes, per batch b:
  sim = keys @ memory.T * 5 / (|keys| |memory|)   [K, S] cosine attention
  att = softmax(sim, axis=S)
  result = att @ memory                            [K, D]
  usage_r = usage + att.sum(K); loc = argmin(usage_r)
  usage_w = usage_r with usage_w[loc] = 1
  memory_w = decay * memory with row loc replaced by content

Full inputs in, full outputs out; internally sharded 8 batches per core
across 8 NeuronCores (batch-parallel, no cross-core communication).

Layout strategy per core (8 batches, S=8192 as 64 tiles of 128):
 - memory tiles [s=128, d=128] DMA'd natural; PE-transposes build memT
   (extracted PSUM->SBUF by ScalarE); sim via col4-packed matmuls
   (keysT stationary [d,8], memT tiles moving) -> [k@32j, s] PSUM blocks;
   PE transpose-back yields att^T [s, t, k]; softmax denominators via
   ones-vector matmul partition sums; result via col4-packed matmuls
   (att^T tiles stationary, memory tiles moving) + selector-matmul fold.
 - argmin via iota/compare with batched gpsimd partition all-reduces.
 - usage_w written transposed to a scratch output, host reassembles.
 - memory_w = decay*mem written in bulk, content rows scattered by one
   indirect DMA at the end.
"""

import numpy as np

B, S, D, K = 64, 8192, 128, 8
NCORES = 8
BPC = B // NCORES  # 8 batches per core
T = S // 128  # 64 s-tiles per batch
SCALE = 5.0
DECAY = np.float32(1.0 - 1.0 / S)
EPS = 1e-30
BIGIDX = 65536.0

_cache = {}


def _build():
    import concourse.bacc as bacc
    import concourse.bass as bass
    import concourse.tile as tile
    from concourse import bass_isa, masks, mybir

    F32 = mybir.dt.float32
    I32 = mybir.dt.int32
    BF16 = mybir.dt.bfloat16
    ALU = mybir.AluOpType
    AXL = mybir.AxisListType
    ACTF = mybir.ActivationFunctionType

    nc = bacc.Bacc("TRN2", target_bir_lowering=False, debug=False, num_devices=NCORES)

    mem_in = nc.dram_tensor("mem_in", [BPC, S, D], F32, kind="ExternalInput").ap()
    usage_in = nc.dram_tensor("usage_in", [BPC, S], F32, kind="ExternalInput").ap()
    keys_in = nc.dram_tensor("keys_in", [BPC, K, D], F32, kind="ExternalInput").ap()
    content_in = nc.dram_tensor("content_in", [BPC, D], F32, kind="ExternalInput").ap()

    memw_out = nc.dram_tensor("memw_out", [BPC, S, D], F32, kind="ExternalOutput").ap()
    uT_out = nc.dram_tensor("uT_out", [BPC, 128, T], F32, kind="ExternalOutput").ap()
    res_out = nc.dram_tensor("res_out", [BPC, K, D], F32, kind="ExternalOutput").ap()

    def bcast_free(ap, counts):
        """Broadcast a [P, 1] (or [P, n]) AP along extra step-0 free dims.
        counts: list of (step, count) pairs replacing the free dims."""
        return bass.AP(ap.tensor, ap.offset, [ap.ap[0]] + [list(c) for c in counts])

    with tile.TileContext(nc) as tc:
        import contextlib

        ctx = contextlib.ExitStack()
        with ctx:
            consts = ctx.enter_context(tc.tile_pool(name="consts", bufs=1))
            big = ctx.enter_context(tc.tile_pool(name="big", bufs=2))
            med = ctx.enter_context(tc.tile_pool(name="med", bufs=2))
            small = ctx.enter_context(tc.tile_pool(name="small", bufs=2))
            stash = ctx.enter_context(tc.tile_pool(name="stash", bufs=1))
            ps_t = ctx.enter_context(tc.tile_pool(name="ps_t", bufs=2, space="PSUM"))
            ps_big = ctx.enter_context(tc.tile_pool(name="ps_big", bufs=1, space="PSUM"))
            ps_aux = ctx.enter_context(tc.tile_pool(name="ps_aux", bufs=2, space="PSUM"))

            # ---------------- constants (inline data) ----------------
            ident_np = np.eye(128, dtype=np.float32)
            p_idx, t_idx = np.mgrid[0:128, 0:T]
            iota_np = (t_idx * 128 + p_idx).astype(np.float32)
            selm_np = (p_idx[:, 0:K] % 32 == t_idx[:, 0:K]).astype(np.float32)
            base_np = (np.arange(K, dtype=np.float32) * S).reshape(K, 1)

            ident = consts.tile([128, 128], F32)
            nc.sync.dma_start(ident[:], nc.inline_tensor(ident_np, name="identc").ap())
            iota_f = consts.tile([128, T], F32)
            nc.sync.dma_start(iota_f[:], nc.inline_tensor(iota_np, name="iotac").ap())
            iota_m = consts.tile([128, T], F32)
            nc.sync.dma_start(
                iota_m[:], nc.inline_tensor(iota_np - BIGIDX, name="iotamc").ap()
            )
            selm = consts.tile([128, K], F32)
            nc.sync.dma_start(selm[:], nc.inline_tensor(selm_np, name="selmc").ap())
            base_f = consts.tile([K, 1], F32)
            nc.sync.dma_start(base_f[:], nc.inline_tensor(base_np, name="basec").ap())

            onescol = consts.tile([128, 1], F32)
            nc.vector.memset(onescol[:], 1.0)
            decayc = consts.tile([128, 1], F32)
            nc.vector.memset(decayc[:], float(DECAY))
            bigc = consts.tile([128, 1], F32)
            nc.vector.memset(bigc[:], BIGIDX)

            # content rows for final scatter
            content_sb = consts.tile([BPC, D], F32)
            nc.sync.dma_start(content_sb[:], content_in[:])

            # PE warmup (overlaps first DMA): bf16 matmuls on identity
            wps = ps_aux.tile([128, 256], F32, name="warm", tag="aux")
            for i in range(40):
                nc.tensor.matmul(
                    wps[:],
                    ident.bitcast(BF16)[:, 0:128],
                    ident.bitcast(BF16)[:, 0:256],
                    start=(i == 0),
                    stop=(i == 39),
                )

            # ---------------- keys prep ----------------
            keys_sb = consts.tile([B // NCORES * K, D], F32)  # [64, 128]
            nc.sync.dma_start(keys_sb[:], keys_in.rearrange("b k d -> (b k) d"))
            ksq = consts.tile([64, D], F32)
            nc.vector.tensor_tensor(out=ksq[:], in0=keys_sb[:], in1=keys_sb[:], op=ALU.mult)
            kss = consts.tile([64, 1], F32)
            nc.vector.tensor_reduce(out=kss[:], in_=ksq[:], axis=AXL.X, op=ALU.add)
            knorm = consts.tile([64, 1], F32)
            nc.scalar.activation(out=knorm[:], in_=kss[:], func=ACTF.Sqrt)
            kinv = consts.tile([64, 1], F32)
            nc.vector.reciprocal(out=kinv[:], in_=knorm[:])
            # NR refine: kinv = kinv*(2 - knorm*kinv)
            knr = consts.tile([64, 1], F32)
            nc.vector.tensor_tensor(out=knr[:], in0=knorm[:], in1=kinv[:], op=ALU.mult)
            nc.vector.tensor_scalar(
                out=knr[:], in0=knr[:], scalar1=-1.0, scalar2=2.0, op0=ALU.mult, op1=ALU.add
            )
            nc.vector.tensor_tensor(out=kinv[:], in0=kinv[:], in1=knr[:], op=ALU.mult)
            # kscale = 5 * kinv ; keys_scaled = keys * kscale (per-partition)
            nc.vector.tensor_scalar(
                out=kinv[:], in0=kinv[:], scalar1=SCALE, scalar2=None, op0=ALU.mult
            )
            keys_sc = consts.tile([64, D], F32)
            nc.scalar.activation(out=keys_sc[:], in_=keys_sb[:], func=ACTF.Copy, scale=kinv[:])
            # transpose -> keysT [128 d, 64 bk]
            kps = ps_aux.tile([128, 64], F32, name="kps", tag="aux")
            nc.tensor.transpose(kps[:], keys_sc[:], ident[0:64, 0:64])
            keysT = consts.tile([128, 64], F32)
            nc.vector.tensor_copy(keysT[:], kps[:])

            # ---------------- per-batch stashes ----------------
            usageT_all = stash.tile([128, BPC, T], F32)
            usager_all = stash.tile([128, BPC, T], F32)
            rowminneg = stash.tile([128, BPC], F32)
            candneg = stash.tile([128, BPC], F32)
            result_sb = stash.tile([K, BPC, D], F32)

            memw_dmas = []

            # ---------------- main batch loop ----------------
            for b in range(BPC):
                mem = big.tile([128, T, D], F32, name="mem")  # [p, t, d]
                nc.sync.dma_start(
                    mem[:], mem_in[b].rearrange("(t p) d -> p t d", p=128)
                )
                urows = small.tile([T, 128], F32, name="urows")
                nc.sync.dma_start(urows[:], usage_in[b].rearrange("(t p) -> t p", p=128))

                # --- transposes: memT tiles [d, s] ---
                memT = big.tile([128, T, D], F32, name="memT")
                for g in range(8):  # groups of 8 tiles
                    tp = ps_t.tile([128, 8, 128], F32, name="tp")
                    for j in range(8):
                        t = g * 8 + j
                        nc.tensor.transpose(tp[:, j, :], mem[:, t, :], ident[:])
                    nc.scalar.copy(memT[:, g * 8 : g * 8 + 8, :], tp[:])

                # --- norms: ss[p, t] = sum_d mem^2 ---
                ss = med.tile([128, T], F32, name="ss")
                for c in range(4):
                    sq = med.tile([128, 16, D], F32, name="sq")
                    nc.vector.tensor_tensor(
                        out=sq[:],
                        in0=mem[:, 16 * c : 16 * c + 16, :],
                        in1=mem[:, 16 * c : 16 * c + 16, :],
                        op=ALU.mult,
                    )
                    nc.vector.tensor_reduce(
                        out=ss[:, 16 * c : 16 * c + 16], in_=sq[:], axis=AXL.X, op=ALU.add
                    )
                # meminv = 1/(eps + sqrt(ss)), NR-refined against rsqrt
                mnorm = med.tile([128, T], F32, name="mnorm")
                nc.scalar.activation(out=mnorm[:], in_=ss[:], func=ACTF.Sqrt)
                minv = med.tile([128, T], F32, name="minv")
                nc.vector.reciprocal(out=minv[:], in_=mnorm[:])
                mnr = med.tile([128, T], F32, name="mnr")
                nc.vector.tensor_tensor(out=mnr[:], in0=mnorm[:], in1=minv[:], op=ALU.mult)
                nc.vector.tensor_scalar(
                    out=mnr[:], in0=mnr[:], scalar1=-1.0, scalar2=2.0,
                    op0=ALU.mult, op1=ALU.add,
                )
                nc.vector.tensor_tensor(out=minv[:], in0=minv[:], in1=mnr[:], op=ALU.mult)

                # --- sim + transpose-back + exp, in two halves of 32 tiles ---
                att = med.tile([128, T, K], F32, name="att")  # compact [s, t, k]
                for h in range(2):
                    simp = ps_big.tile([128, 8, 128], F32, name="pbig")
                    for g in range(8):  # 8 blocks of 4 tiles
                        for j in range(4):
                            t = h * 32 + g * 4 + j
                            nc.tensor.matmul(
                                simp[32 * j : 32 * j + 8, g, :],
                                keysT[:, 8 * b : 8 * b + 8],
                                memT[:, t, :],
                                tile_position=(0, 32 * j),
                            )
                    sim_sb = med.tile([128, 8, 128], F32, name="sim_sb")
                    nc.vector.tensor_copy(sim_sb[:], simp[:])
                    tbp = ps_big.tile([128, 8, 128], F32, name="pbig")
                    for g in range(8):
                        nc.tensor.transpose(tbp[:, g, :], sim_sb[:, g, :], ident[:])
                    # scaled extract (valid cols only): att_raw = tbp * minv[t]
                    # tbp cols: (g, 32j + k) -> t = h*32 + 4g + j
                    tbp_v = tbp.rearrange("p g (j x) -> p g j x", j=4)[:, :, :, 0:K]
                    minv_b = bass.AP(
                        minv.tensor,
                        minv.offset + h * 32,
                        [minv.ap[0], [4, 8], [1, 4], [0, K]],
                    )
                    nc.vector.tensor_tensor(
                        out=att[:, h * 32 : h * 32 + 32, :].rearrange(
                            "p (g j) k -> p g j k", j=4
                        ),
                        in0=tbp_v,
                        in1=minv_b,
                        op=ALU.mult,
                    )
                nc.scalar.activation(
                    out=att.rearrange("p t k -> p (t k)"),
                    in_=att.rearrange("p t k -> p (t k)"),
                    func=ACTF.Exp,
                )

                # --- softmax denominators: colsum over s-partitions and t ---
                osum = ps_aux.tile([1, T * K], F32, name="osum", tag="aux")
                nc.tensor.matmul(
                    osum[:], onescol[:], att.rearrange("p t k -> p (t k)")
                )
                sums = small.tile([1, K], F32, name="sums")
                nc.vector.tensor_reduce(
                    out=sums[:],
                    in_=bass.AP(
                        osum.tensor, osum.offset, [osum.ap[0], [1, K], [K, T]]
                    ),
                    axis=AXL.X,
                    op=ALU.add,
                )
                inv = small.tile([1, K], F32, name="inv")
                nc.vector.reciprocal(out=inv[:], in_=sums[:])
                snr = small.tile([1, K], F32, name="snr")
                nc.vector.tensor_tensor(out=snr[:], in0=sums[:], in1=inv[:], op=ALU.mult)
                nc.vector.tensor_scalar(
                    out=snr[:], in0=snr[:], scalar1=-1.0, scalar2=2.0,
                    op0=ALU.mult, op1=ALU.add,
                )
                nc.vector.tensor_tensor(out=inv[:], in0=inv[:], in1=snr[:], op=ALU.mult)
                invb = small.tile([128, K], F32, name="invb")
                nc.gpsimd.partition_broadcast(invb[:], inv[:])
                # inv8: [8, 1] per-k for result scaling (transpose of inv row)
                ivp = ps_aux.tile([K, 1], F32, name="ivp", tag="aux")
                nc.tensor.transpose(ivp[:], inv[:], ident[0:1, 0:1])
                inv8 = small.tile([K, 1], F32, name="inv8")
                nc.vector.tensor_copy(inv8[:], ivp[:])
                # att_norm (separate tile; unnormalized att feeds result matmuls)
                attn = med.tile([128, T, K], F32, name="attn")
                nc.vector.tensor_tensor(
                    out=attn[:],
                    in0=att[:],
                    in1=bcast_free(invb, [(0, T), (invb.ap[1][0], K)]),
                    op=ALU.mult,
                )

                # --- usage_r and argmin prep ---
                ksum = med.tile([128, T], F32, name="ksum")
                nc.vector.tensor_reduce(out=ksum[:], in_=attn[:], axis=AXL.X, op=ALU.add)
                up = ps_aux.tile([128, T], F32, name="up", tag="aux")
                nc.tensor.transpose(up[:, 0:64], urows[:], ident[0:64, 0:64])
                usageT = usageT_all[:, b, :]
                nc.vector.tensor_copy(usageT, up[:, 0:64])
                usager = usager_all[:, b, :]
                nc.vector.tensor_tensor(out=usager, in0=usageT, in1=ksum[:], op=ALU.add)
                rmn = small.tile([128, 1], F32, name="rmn")
                nc.vector.tensor_reduce(out=rmn[:], in_=usager, axis=AXL.X, op=ALU.min)
                nc.vector.tensor_scalar(
                    out=rowminneg[:, b : b + 1], in0=rmn[:], scalar1=-1.0,
                    scalar2=None, op0=ALU.mult,
                )

                # --- result: col4 matmuls, accumulate over t ---
                rps = ps_aux.tile([128, 128], F32, name="rps", tag="aux")
                for t in range(T):
                    j = t % 4
                    nc.tensor.matmul(
                        rps[32 * j : 32 * j + 8, :],
                        att[:, t, :],
                        mem[:, t, :],
                        start=(t < 4),
                        stop=(t >= T - 4),
                        tile_position=(0, 32 * j),
                    )
                res_blk = small.tile([128, 128], F32, name="res_blk")
                nc.vector.tensor_copy(res_blk[:], rps[:])
                fps = ps_aux.tile([K, 128], F32, name="fps", tag="aux")
                nc.tensor.matmul(fps[:], selm[:], res_blk[:])
                nc.scalar.activation(
                    out=result_sb[:, b, :], in_=fps[:], func=ACTF.Copy, scale=inv8[:]
                )

                # --- decay in place + bulk writeback ---
                for c in range(4):
                    nc.vector.tensor_scalar(
                        out=mem[:, 16 * c : 16 * c + 16, :],
                        in0=mem[:, 16 * c : 16 * c + 16, :],
                        scalar1=decayc[:],
                        scalar2=None,
                        op0=ALU.mult,
                    )
                dma = nc.scalar.dma_start(
                    memw_out[b].rearrange("(t p) d -> p t d", p=128), mem[:]
                )
                memw_dmas.append(dma)

            # ---------------- argmin finale ----------------
            # partition-max of rowminneg via PE transpose + free-dim reduce
            onesrow = consts.tile([1, 128], F32)
            nc.vector.memset(onesrow[:], 1.0)

            def partition_max_bcast(stash_pm, tag):
                # stash_pm [128, BPC] -> out [128, BPC] columns all equal to
                # per-batch max over partitions; also returns [BPC, 1] row form
                tps_ = ps_aux.tile([BPC, 128], F32, name=f"pm_{tag}", tag="aux")
                nc.tensor.transpose(tps_[:], stash_pm[:], ident[:])
                mrow = small.tile([BPC, 1], F32, name=f"mr_{tag}")
                nc.vector.tensor_reduce(out=mrow[:], in_=tps_[:], axis=AXL.X, op=ALU.max)
                rps_ = ps_aux.tile([1, BPC], F32, name=f"pr_{tag}", tag="aux")
                nc.tensor.transpose(rps_[:], mrow[:], ident[0:BPC, 0:BPC])
                row = small.tile([1, BPC], F32, name=f"rw_{tag}")
                nc.vector.tensor_copy(row[:], rps_[:])
                bps_ = ps_aux.tile([128, BPC], F32, name=f"pb_{tag}", tag="aux")
                nc.tensor.matmul(bps_[:], onesrow[:], row[:])
                out = stash.tile([128, BPC], F32, name=f"bc_{tag}")
                nc.vector.tensor_copy(out[:], bps_[:])
                return out, mrow

            ar1, _ = partition_max_bcast(rowminneg, "g")
            for b in range(BPC):
                gmin = small.tile([128, 1], F32, name="gmin")
                nc.vector.tensor_scalar(
                    out=gmin[:], in0=ar1[:, b : b + 1], scalar1=-1.0,
                    scalar2=None, op0=ALU.mult,
                )
                msk = med.tile([128, T], F32, name="msk")
                nc.vector.tensor_scalar(
                    out=msk[:], in0=usager_all[:, b, :], scalar1=gmin[:],
                    scalar2=None, op0=ALU.is_equal,
                )
                cand = med.tile([128, T], F32, name="cand")
                nc.vector.tensor_tensor(out=cand[:], in0=msk[:], in1=iota_m[:], op=ALU.mult)
                nc.vector.tensor_tensor(
                    out=cand[:], in0=cand[:],
                    in1=bcast_free(bigc, [(0, T)]), op=ALU.add,
                )
                cmn = small.tile([128, 1], F32, name="cmn")
                nc.vector.tensor_reduce(out=cmn[:], in_=cand[:], axis=AXL.X, op=ALU.min)
                nc.vector.tensor_scalar(
                    out=candneg[:, b : b + 1], in0=cmn[:], scalar1=-1.0,
                    scalar2=None, op0=ALU.mult,
                )
            ar2, negrow = partition_max_bcast(candneg, "s")
            # s* per batch (f32): sstar = -ar2 ; s8 = -negrow  [BPC, 1]
            sstar = stash.tile([128, BPC], F32)
            nc.vector.tensor_scalar(
                out=sstar[:], in0=ar2[:], scalar1=-1.0, scalar2=None, op0=ALU.mult
            )
            s8 = stash.tile([BPC, 1], F32)
            nc.vector.tensor_scalar(
                out=s8[:], in0=negrow[:], scalar1=-1.0, scalar2=None, op0=ALU.mult
            )

            # usage_w fixup + write out (transposed scratch)
            for b in range(BPC):
                m2 = med.tile([128, T], mybir.dt.uint8, name="m2")
                nc.vector.tensor_scalar(
                    out=m2[:], in0=iota_f[:], scalar1=sstar[:, b : b + 1],
                    scalar2=None, op0=ALU.is_equal,
                )
                uf = med.tile([128, T], F32, name="uf")
                nc.vector.select(
                    out=uf[:], mask=m2[:],
                    on_true=bcast_free(onescol, [(0, T)]),
                    on_false=usager_all[:, b, :],
                )
                nc.scalar.dma_start(uT_out[b], uf[:])

            # result out
            nc.sync.dma_start(
                res_out.rearrange("b k d -> k b d"), result_sb[:]
            )

            # content scatter: global row idx = b*S + s*
            grow = stash.tile([K, 1], F32)
            nc.vector.tensor_tensor(out=grow[:], in0=s8[:], in1=base_f[:], op=ALU.add)
            gidx = stash.tile([K, 1], I32)
            nc.vector.tensor_copy(gidx[:], grow[:])
            scat = nc.gpsimd.indirect_dma_start(
                out=memw_out.rearrange("b s d -> (b s) d"),
                out_offset=bass.IndirectOffsetOnAxis(ap=gidx[:, 0:1], axis=0),
                in_=content_sb[:],
                in_offset=None,
            )
            for dma in memw_dmas:
                tile.add_dep_helper(scat.ins, dma.ins, sync=True)

    nc.compile()
    return nc


def _get_nc():
    if "nc" not in _cache:
        _cache["nc"] = _build()
    return _cache["nc"]


def kernel(memory, usage, keys, content):
    from concourse.bass_utils import run_bass_kernel_spmd

    nc = _get_nc()
    memory = np.ascontiguousarray(memory, dtype=np.float32)
    usage = np.ascontiguousarray(usage, dtype=np.float32)
    keys = np.ascontiguousarray(keys, dtype=np.float32)
    content = np.ascontiguousarray(content, dtype=np.float32)

    in_maps = []
    for c in range(NCORES):
        sl = slice(c * BPC, (c + 1) * BPC)
        in_maps.append(
            {
                "mem_in": memory[sl],
                "usage_in": usage[sl],
                "keys_in": keys[sl],
                "content_in": content[sl],
            }
        )
    res = run_bass_kernel_spmd(nc, in_maps, core_ids=list(range(NCORES)))
    _cache["last_result"] = res

    result = np.concatenate([r["res_out"] for r in res.results], axis=0)
    memory_w = np.concatenate([r["memw_out"] for r in res.results], axis=0)
    # uT_out[b] is [128(p), 64(t)]; usage_w[b, t*128+p] = uT[p, t]
    usage_w = np.concatenate(
        [r["uT_out"].transpose(0, 2, 1).reshape(BPC, S) for r in res.results], axis=0
    )
    return result, memory_w, usage_w


# revision 8
# speedup vs baseline: 1.1552x; 1.1437x over previous
"""Trainium2 Bass kernel for MemoryWithUsage (scatter_memory).

Computes, per batch b:
  sim = keys @ memory.T * 5 / (|keys| |memory|)   [K, S] cosine attention
  att = softmax(sim, axis=S)
  result = att @ memory                            [K, D]
  usage_r = usage + att.sum(K); loc = argmin(usage_r)
  usage_w = usage_r with usage_w[loc] = 1
  memory_w = decay * memory with row loc replaced by content

Full inputs in, full outputs out; internally sharded 8 batches per core
across 8 NeuronCores (batch-parallel, no cross-core communication).

Layout strategy per core (8 batches, S=8192 as 64 tiles of 128):
 - memory tiles [s=128, d=128] DMA'd natural; PE-transposes build memT
   (extracted PSUM->SBUF by ScalarE); sim via col4-packed matmuls
   (keysT stationary [d,8], memT tiles moving) -> [k@32j, s] PSUM blocks;
   PE transpose-back yields att^T [s, t, k]; softmax denominators via
   ones-vector matmul partition sums; result via col4-packed matmuls
   (att^T tiles stationary, memory tiles moving) + selector-matmul fold.
 - argmin via iota/compare with batched gpsimd partition all-reduces.
 - usage_w written transposed to a scratch output, host reassembles.
 - memory_w = decay*mem written in bulk, content rows scattered by one
   indirect DMA at the end.
"""

import numpy as np

B, S, D, K = 64, 8192, 128, 8
NCORES = 8
BPC = B // NCORES  # 8 batches per core
T = S // 128  # 64 s-tiles per batch
SCALE = 5.0
DECAY = np.float32(1.0 - 1.0 / S)
EPS = 1e-30
BIGIDX = 65536.0

_cache = {}


def _build():
    import concourse.bacc as bacc
    import concourse.bass as bass
    import concourse.tile as tile
    from concourse import bass_isa, masks, mybir

    F32 = mybir.dt.float32
    I32 = mybir.dt.int32
    BF16 = mybir.dt.bfloat16
    ALU = mybir.AluOpType
    AXL = mybir.AxisListType
    ACTF = mybir.ActivationFunctionType

    nc = bacc.Bacc("TRN2", target_bir_lowering=False, debug=False, num_devices=NCORES)

    mem_in = nc.dram_tensor("mem_in", [BPC, S, D], F32, kind="ExternalInput").ap()
    usage_in = nc.dram_tensor("usage_in", [BPC, S], F32, kind="ExternalInput").ap()
    keys_in = nc.dram_tensor("keys_in", [BPC, K, D], F32, kind="ExternalInput").ap()
    content_in = nc.dram_tensor("content_in", [BPC, D], F32, kind="ExternalInput").ap()

    memw_out = nc.dram_tensor("memw_out", [BPC, S, D], F32, kind="ExternalOutput").ap()
    uT_out = nc.dram_tensor("uT_out", [BPC, 128, T], F32, kind="ExternalOutput").ap()
    res_out = nc.dram_tensor("res_out", [BPC, K, D], F32, kind="ExternalOutput").ap()

    def bcast_free(ap, counts):
        """Broadcast a [P, 1] (or [P, n]) AP along extra step-0 free dims.
        counts: list of (step, count) pairs replacing the free dims."""
        return bass.AP(ap.tensor, ap.offset, [ap.ap[0]] + [list(c) for c in counts])

    with tile.TileContext(nc) as tc:
        import contextlib

        ctx = contextlib.ExitStack()
        with ctx:
            consts = ctx.enter_context(tc.tile_pool(name="consts", bufs=1))
            big = ctx.enter_context(tc.tile_pool(name="big", bufs=2))
            med = ctx.enter_context(tc.tile_pool(name="med", bufs=2))
            small = ctx.enter_context(tc.tile_pool(name="small", bufs=2))
            stash = ctx.enter_context(tc.tile_pool(name="stash", bufs=1))
            ps_t = ctx.enter_context(tc.tile_pool(name="ps_t", bufs=2, space="PSUM"))
            ps_big = ctx.enter_context(tc.tile_pool(name="ps_big", bufs=1, space="PSUM"))
            ps_aux = ctx.enter_context(tc.tile_pool(name="ps_aux", bufs=2, space="PSUM"))

            # ---------------- constants (inline data) ----------------
            ident_np = np.eye(128, dtype=np.float32)
            p_idx, t_idx = np.mgrid[0:128, 0:T]
            iota_np = (p_idx * 64 + t_idx).astype(np.float32)
            selm_np = (p_idx[:, 0:K] % 32 == t_idx[:, 0:K]).astype(np.float32)
            base_np = (np.arange(K, dtype=np.float32) * S).reshape(K, 1)

            ident = consts.tile([128, 128], F32)
            nc.sync.dma_start(ident[:], nc.inline_tensor(ident_np, name="identc").ap())
            iota_f = consts.tile([128, T], F32)
            nc.sync.dma_start(iota_f[:], nc.inline_tensor(iota_np, name="iotac").ap())
            iota_m = consts.tile([128, T], F32)
            nc.sync.dma_start(
                iota_m[:], nc.inline_tensor(iota_np - BIGIDX, name="iotamc").ap()
            )
            selm = consts.tile([128, K], F32)
            nc.sync.dma_start(selm[:], nc.inline_tensor(selm_np, name="selmc").ap())
            base_f = consts.tile([K, 1], F32)
            nc.sync.dma_start(base_f[:], nc.inline_tensor(base_np, name="basec").ap())

            onescol = consts.tile([128, 1], F32)
            nc.vector.memset(onescol[:], 1.0)
            decayc = consts.tile([128, 1], F32)
            nc.vector.memset(decayc[:], float(DECAY))
            bigc = consts.tile([128, 1], F32)
            nc.vector.memset(bigc[:], BIGIDX)

            # content rows for final scatter
            content_sb = consts.tile([BPC, D], F32)
            nc.sync.dma_start(content_sb[:], content_in[:])

            # PE warmup (overlaps first DMA): bf16 matmuls on identity
            wps = ps_aux.tile([128, 256], F32, name="warm", tag="aux")
            for i in range(40):
                nc.tensor.matmul(
                    wps[:],
                    ident.bitcast(BF16)[:, 0:128],
                    ident.bitcast(BF16)[:, 0:256],
                    start=(i == 0),
                    stop=(i == 39),
                )

            # ---------------- keys prep ----------------
            keys_sb = consts.tile([B // NCORES * K, D], F32)  # [64, 128]
            nc.sync.dma_start(keys_sb[:], keys_in.rearrange("b k d -> (b k) d"))
            ksq = consts.tile([64, D], F32)
            nc.vector.tensor_tensor(out=ksq[:], in0=keys_sb[:], in1=keys_sb[:], op=ALU.mult)
            kss = consts.tile([64, 1], F32)
            nc.vector.tensor_reduce(out=kss[:], in_=ksq[:], axis=AXL.X, op=ALU.add)
            knorm = consts.tile([64, 1], F32)
            nc.scalar.activation(out=knorm[:], in_=kss[:], func=ACTF.Sqrt)
            kinv = consts.tile([64, 1], F32)
            nc.vector.reciprocal(out=kinv[:], in_=knorm[:])
            # NR refine: kinv = kinv*(2 - knorm*kinv)
            knr = consts.tile([64, 1], F32)
            nc.vector.tensor_tensor(out=knr[:], in0=knorm[:], in1=kinv[:], op=ALU.mult)
            nc.vector.tensor_scalar(
                out=knr[:], in0=knr[:], scalar1=-1.0, scalar2=2.0, op0=ALU.mult, op1=ALU.add
            )
            nc.vector.tensor_tensor(out=kinv[:], in0=kinv[:], in1=knr[:], op=ALU.mult)
            # kscale = 5 * kinv ; keys_scaled = keys * kscale (per-partition)
            nc.vector.tensor_scalar(
                out=kinv[:], in0=kinv[:], scalar1=SCALE, scalar2=None, op0=ALU.mult
            )
            keys_sc = consts.tile([64, D], F32)
            nc.scalar.activation(out=keys_sc[:], in_=keys_sb[:], func=ACTF.Copy, scale=kinv[:])
            # transpose -> keysT [128 d, 64 bk]
            kps = ps_aux.tile([128, 64], F32, name="kps", tag="aux")
            nc.tensor.transpose(kps[:], keys_sc[:], ident[0:64, 0:64])
            keysT = consts.tile([128, 64], F32)
            nc.vector.tensor_copy(keysT[:], kps[:])

            # ---------------- per-batch stashes ----------------
            usageT_all = stash.tile([128, BPC, T], F32)
            usager_all = stash.tile([128, BPC, T], F32)
            rowminneg = stash.tile([128, BPC], F32)
            candneg = stash.tile([128, BPC], F32)
            result_sb = stash.tile([K, BPC, D], F32)

            memw_dmas = []

            # ---------------- main batch loop ----------------
            for b in range(BPC):
                mem = big.tile([128, T, D], F32, name="mem")  # [p, t, d]
                nc.sync.dma_start(
                    mem[:], mem_in[b].rearrange("(p t) d -> p t d", p=128)
                )

                # --- transposes: memT tiles [d, s] ---
                memT = big.tile([128, T, D], F32, name="memT")
                for g in range(8):  # groups of 8 tiles
                    tp = ps_t.tile([128, 8, 128], F32, name="tp")
                    for j in range(8):
                        t = g * 8 + j
                        nc.tensor.transpose(tp[:, j, :], mem[:, t, :], ident[:])
                    nc.scalar.copy(memT[:, g * 8 : g * 8 + 8, :], tp[:])

                # --- norms: ss[p, t] = sum_d mem^2 ---
                ss = med.tile([128, T], F32, name="ss")
                for c in range(4):
                    sq = med.tile([128, 16, D], F32, name="sq")
                    nc.vector.tensor_tensor(
                        out=sq[:],
                        in0=mem[:, 16 * c : 16 * c + 16, :],
                        in1=mem[:, 16 * c : 16 * c + 16, :],
                        op=ALU.mult,
                    )
                    nc.vector.tensor_reduce(
                        out=ss[:, 16 * c : 16 * c + 16], in_=sq[:], axis=AXL.X, op=ALU.add
                    )
                # meminv = 1/(eps + sqrt(ss)), NR-refined against rsqrt
                mnorm = med.tile([128, T], F32, name="mnorm")
                nc.scalar.activation(out=mnorm[:], in_=ss[:], func=ACTF.Sqrt)
                minv = med.tile([128, T], F32, name="minv")
                nc.vector.reciprocal(out=minv[:], in_=mnorm[:])
                mnr = med.tile([128, T], F32, name="mnr")
                nc.vector.tensor_tensor(out=mnr[:], in0=mnorm[:], in1=minv[:], op=ALU.mult)
                nc.vector.tensor_scalar(
                    out=mnr[:], in0=mnr[:], scalar1=-1.0, scalar2=2.0,
                    op0=ALU.mult, op1=ALU.add,
                )
                nc.vector.tensor_tensor(out=minv[:], in0=minv[:], in1=mnr[:], op=ALU.mult)

                # --- sim + transpose-back + exp, in two halves of 32 tiles ---
                att = med.tile([128, T, K], F32, name="att")  # compact [s, t, k]
                for h in range(2):
                    simp = ps_big.tile([128, 8, 128], F32, name="pbig")
                    for g in range(8):  # 8 blocks of 4 tiles
                        for j in range(4):
                            t = h * 32 + g * 4 + j
                            nc.tensor.matmul(
                                simp[32 * j : 32 * j + 8, g, :],
                                keysT[:, 8 * b : 8 * b + 8],
                                memT[:, t, :],
                                tile_position=(0, 32 * j),
                            )
                    sim_sb = med.tile([128, 8, 128], F32, name="sim_sb")
                    nc.vector.tensor_copy(sim_sb[:], simp[:])
                    tbp = ps_big.tile([128, 8, 128], F32, name="pbig")
                    for g in range(8):
                        nc.tensor.transpose(tbp[:, g, :], sim_sb[:, g, :], ident[:])
                    # scaled extract (valid cols only): att_raw = tbp * minv[t]
                    # tbp cols: (g, 32j + k) -> t = h*32 + 4g + j
                    tbp_v = tbp.rearrange("p g (j x) -> p g j x", j=4)[:, :, :, 0:K]
                    minv_b = bass.AP(
                        minv.tensor,
                        minv.offset + h * 32,
                        [minv.ap[0], [4, 8], [1, 4], [0, K]],
                    )
                    nc.vector.tensor_tensor(
                        out=att[:, h * 32 : h * 32 + 32, :].rearrange(
                            "p (g j) k -> p g j k", j=4
                        ),
                        in0=tbp_v,
                        in1=minv_b,
                        op=ALU.mult,
                    )
                nc.scalar.activation(
                    out=att.rearrange("p t k -> p (t k)"),
                    in_=att.rearrange("p t k -> p (t k)"),
                    func=ACTF.Exp,
                )

                # --- softmax denominators: colsum over s-partitions and t ---
                osum = ps_aux.tile([1, T * K], F32, name="osum", tag="aux")
                nc.tensor.matmul(
                    osum[:], onescol[:], att.rearrange("p t k -> p (t k)")
                )
                sums = small.tile([1, K], F32, name="sums")
                nc.vector.tensor_reduce(
                    out=sums[:],
                    in_=bass.AP(
                        osum.tensor, osum.offset, [osum.ap[0], [1, K], [K, T]]
                    ),
                    axis=AXL.X,
                    op=ALU.add,
                )
                inv = small.tile([1, K], F32, name="inv")
                nc.vector.reciprocal(out=inv[:], in_=sums[:])
                snr = small.tile([1, K], F32, name="snr")
                nc.vector.tensor_tensor(out=snr[:], in0=sums[:], in1=inv[:], op=ALU.mult)
                nc.vector.tensor_scalar(
                    out=snr[:], in0=snr[:], scalar1=-1.0, scalar2=2.0,
                    op0=ALU.mult, op1=ALU.add,
                )
                nc.vector.tensor_tensor(out=inv[:], in0=inv[:], in1=snr[:], op=ALU.mult)
                invb = small.tile([128, K], F32, name="invb")
                nc.gpsimd.partition_broadcast(invb[:], inv[:])
                # inv8: [8, 1] per-k for result scaling (transpose of inv row)
                ivp = ps_aux.tile([K, 1], F32, name="ivp", tag="aux")
                nc.tensor.transpose(ivp[:], inv[:], ident[0:1, 0:1])
                inv8 = small.tile([K, 1], F32, name="inv8")
                nc.vector.tensor_copy(inv8[:], ivp[:])
                # att_norm (separate tile; unnormalized att feeds result matmuls)
                attn = med.tile([128, T, K], F32, name="attn")
                nc.vector.tensor_tensor(
                    out=attn[:],
                    in0=att[:],
                    in1=bcast_free(invb, [(0, T), (invb.ap[1][0], K)]),
                    op=ALU.mult,
                )

                # --- usage_r and argmin prep ---
                ksum = med.tile([128, T], F32, name="ksum")
                nc.vector.tensor_reduce(out=ksum[:], in_=attn[:], axis=AXL.X, op=ALU.add)
                usageT = usageT_all[:, b, :]
                nc.sync.dma_start(usageT, usage_in[b].rearrange("(p t) -> p t", p=128))
                usager = usager_all[:, b, :]
                nc.vector.tensor_tensor(out=usager, in0=usageT, in1=ksum[:], op=ALU.add)
                rmn = small.tile([128, 1], F32, name="rmn")
                nc.vector.tensor_reduce(out=rmn[:], in_=usager, axis=AXL.X, op=ALU.min)
                nc.vector.tensor_scalar(
                    out=rowminneg[:, b : b + 1], in0=rmn[:], scalar1=-1.0,
                    scalar2=None, op0=ALU.mult,
                )

                # --- result: col4 matmuls, accumulate over t ---
                rps = ps_aux.tile([128, 128], F32, name="rps", tag="aux")
                for t in range(T):
                    j = t % 4
                    nc.tensor.matmul(
                        rps[32 * j : 32 * j + 8, :],
                        att[:, t, :],
                        mem[:, t, :],
                        start=(t < 4),
                        stop=(t >= T - 4),
                        tile_position=(0, 32 * j),
                    )
                res_blk = small.tile([128, 128], F32, name="res_blk")
                nc.vector.tensor_copy(res_blk[:], rps[:])
                fps = ps_aux.tile([K, 128], F32, name="fps", tag="aux")
                nc.tensor.matmul(fps[:], selm[:], res_blk[:])
                nc.scalar.activation(
                    out=result_sb[:, b, :], in_=fps[:], func=ACTF.Copy, scale=inv8[:]
                )

                # --- decay in place + bulk writeback ---
                for c in range(4):
                    nc.vector.tensor_scalar(
                        out=mem[:, 16 * c : 16 * c + 16, :],
                        in0=mem[:, 16 * c : 16 * c + 16, :],
                        scalar1=decayc[:],
                        scalar2=None,
                        op0=ALU.mult,
                    )
                dma = nc.scalar.dma_start(
                    memw_out[b].rearrange("(p t) d -> p t d", p=128), mem[:]
                )
                memw_dmas.append(dma)

            # ---------------- argmin finale ----------------
            # partition-max of rowminneg via PE transpose + free-dim reduce
            onesrow = consts.tile([1, 128], F32)
            nc.vector.memset(onesrow[:], 1.0)

            def partition_max_bcast(stash_pm, tag):
                # stash_pm [128, BPC] -> out [128, BPC] columns all equal to
                # per-batch max over partitions; also returns [BPC, 1] row form
                tps_ = ps_aux.tile([BPC, 128], F32, name=f"pm_{tag}", tag="aux")
                nc.tensor.transpose(tps_[:], stash_pm[:], ident[:])
                mrow = small.tile([BPC, 1], F32, name=f"mr_{tag}")
                nc.vector.tensor_reduce(out=mrow[:], in_=tps_[:], axis=AXL.X, op=ALU.max)
                rps_ = ps_aux.tile([1, BPC], F32, name=f"pr_{tag}", tag="aux")
                nc.tensor.transpose(rps_[:], mrow[:], ident[0:BPC, 0:BPC])
                row = small.tile([1, BPC], F32, name=f"rw_{tag}")
                nc.vector.tensor_copy(row[:], rps_[:])
                bps_ = ps_aux.tile([128, BPC], F32, name=f"pb_{tag}", tag="aux")
                nc.tensor.matmul(bps_[:], onesrow[:], row[:])
                out = stash.tile([128, BPC], F32, name=f"bc_{tag}")
                nc.vector.tensor_copy(out[:], bps_[:])
                return out, mrow

            ar1, _ = partition_max_bcast(rowminneg, "g")
            for b in range(BPC):
                gmin = small.tile([128, 1], F32, name="gmin")
                nc.vector.tensor_scalar(
                    out=gmin[:], in0=ar1[:, b : b + 1], scalar1=-1.0,
                    scalar2=None, op0=ALU.mult,
                )
                msk = med.tile([128, T], F32, name="msk")
                nc.vector.tensor_scalar(
                    out=msk[:], in0=usager_all[:, b, :], scalar1=gmin[:],
                    scalar2=None, op0=ALU.is_equal,
                )
                cand = med.tile([128, T], F32, name="cand")
                nc.vector.tensor_tensor(out=cand[:], in0=msk[:], in1=iota_m[:], op=ALU.mult)
                nc.vector.tensor_tensor(
                    out=cand[:], in0=cand[:],
                    in1=bcast_free(bigc, [(0, T)]), op=ALU.add,
                )
                cmn = small.tile([128, 1], F32, name="cmn")
                nc.vector.tensor_reduce(out=cmn[:], in_=cand[:], axis=AXL.X, op=ALU.min)
                nc.vector.tensor_scalar(
                    out=candneg[:, b : b + 1], in0=cmn[:], scalar1=-1.0,
                    scalar2=None, op0=ALU.mult,
                )
            ar2, negrow = partition_max_bcast(candneg, "s")
            # s* per batch (f32): sstar = -ar2 ; s8 = -negrow  [BPC, 1]
            sstar = stash.tile([128, BPC], F32)
            nc.vector.tensor_scalar(
                out=sstar[:], in0=ar2[:], scalar1=-1.0, scalar2=None, op0=ALU.mult
            )
            s8 = stash.tile([BPC, 1], F32)
            nc.vector.tensor_scalar(
                out=s8[:], in0=negrow[:], scalar1=-1.0, scalar2=None, op0=ALU.mult
            )

            # usage_w fixup + write out (transposed scratch)
            for b in range(BPC):
                m2 = med.tile([128, T], mybir.dt.uint8, name="m2")
                nc.vector.tensor_scalar(
                    out=m2[:], in0=iota_f[:], scalar1=sstar[:, b : b + 1],
                    scalar2=None, op0=ALU.is_equal,
                )
                uf = med.tile([128, T], F32, name="uf")
                nc.vector.select(
                    out=uf[:], mask=m2[:],
                    on_true=bcast_free(onescol, [(0, T)]),
                    on_false=usager_all[:, b, :],
                )
                nc.scalar.dma_start(uT_out[b], uf[:])

            # result out
            nc.sync.dma_start(
                res_out.rearrange("b k d -> k b d"), result_sb[:]
            )

            # content scatter: global row idx = b*S + s*
            grow = stash.tile([K, 1], F32)
            nc.vector.tensor_tensor(out=grow[:], in0=s8[:], in1=base_f[:], op=ALU.add)
            gidx = stash.tile([K, 1], I32)
            nc.vector.tensor_copy(gidx[:], grow[:])
            scat = nc.gpsimd.indirect_dma_start(
                out=memw_out.rearrange("b s d -> (b s) d"),
                out_offset=bass.IndirectOffsetOnAxis(ap=gidx[:, 0:1], axis=0),
                in_=content_sb[:],
                in_offset=None,
            )
            for dma in memw_dmas:
                tile.add_dep_helper(scat.ins, dma.ins, sync=True)

    nc.compile()
    return nc


def _get_nc():
    if "nc" not in _cache:
        _cache["nc"] = _build()
    return _cache["nc"]


def kernel(memory, usage, keys, content):
    from concourse.bass_utils import run_bass_kernel_spmd

    nc = _get_nc()
    memory = np.ascontiguousarray(memory, dtype=np.float32)
    usage = np.ascontiguousarray(usage, dtype=np.float32)
    keys = np.ascontiguousarray(keys, dtype=np.float32)
    content = np.ascontiguousarray(content, dtype=np.float32)

    in_maps = []
    for c in range(NCORES):
        sl = slice(c * BPC, (c + 1) * BPC)
        in_maps.append(
            {
                "mem_in": memory[sl],
                "usage_in": usage[sl],
                "keys_in": keys[sl],
                "content_in": content[sl],
            }
        )
    res = run_bass_kernel_spmd(nc, in_maps, core_ids=list(range(NCORES)))
    _cache["last_result"] = res

    result = np.concatenate([r["res_out"] for r in res.results], axis=0)
    memory_w = np.concatenate([r["memw_out"] for r in res.results], axis=0)
    # uT_out[b] is [128(p), 64(t)] with s = p*64 + t (p-major): plain reshape
    usage_w = np.concatenate(
        [r["uT_out"].reshape(BPC, S) for r in res.results], axis=0
    )
    return result, memory_w, usage_w
